# revision 22
# baseline (speedup 1.0000x reference)
"""Multi-head self-attention Trainium2 kernel (8 NeuronCores).

Problem: B=4, N=2048, D=1024, H=16 heads of dim 64, fp32 in/out.

Sharding: 8 cores = 4 batches x 2 head-groups. Core c handles batch c//2
and heads (c%2)*8 .. (c%2)*8+7 (a 512-wide slice of the hidden dim).
Each core computes q/k/v projections for its head slice, attention for
its 8 heads, and a partial out-projection (contraction over its 512
attention dims). Host sums the two partials per batch.

Device dataflow (per core), all matmuls bf16 with fp32 PSUM accumulate:
  - x^T (host-pretransposed, bf16) in SBUF as 8 [128, 2048] tiles.
  - q_a/k_a in "layout a" [head_dim-part, token-free]; v in "layout b"
    [token-part, head_dim-free] restrided into per-head 65-col segments
    whose last column is ones (softmax denominator falls out of PV).
  - scores transposed: S^T[j, i] = k_a^T q_a (K=64), exp on ScalarE
    (scale 1/8 folded in; scores ~N(0,1) so no max subtraction), P^T
    bf16 straight to SBUF; PV accumulates [65, i] over 16 j-tiles.

Scheduling (the part that matters for speed): the PE p-state ramp
makes any tensor-engine idle gap doubly expensive (the next ~3us run
at half clock), and the ScalarE exp stream (33.5M elem -> ~280us) is
nearly as long as all matmul streams together (~330us), so the two
must overlap continuously. The kernel therefore runs 16 single-head
attention units (8 heads x 2 i-phases of 1024 tokens) sized so PSUM
splits into: scores [128,1024] x2 bufs (4 banks) + PV accum [65,1024]
x1 (2 banks) + a dedicated 2-bank "filler" pool. All remaining
projection / out-projection matmuls are emitted as fine-grained
fillers between attention ops, keeping the PE dense while ScalarE
holds a small exp lead. Input DMAs are ordered so the first projection
matmul can issue ~2us in (k-outer loops consume tiles as they land).

Normalize: reciprocal_approx_fast on the fp32 PV copy's denominator
row, partition-broadcast, multiply - off the critical path. Out-proj
for the last 8 token tiles accumulates pairs {0,1,2} early into bf16
partials; only the final pair's matmuls trail the last normalize.

Biases: bq on device; bk cancels in softmax; bv/bo folded on host.
"""

import numpy as np
import ml_dtypes
from collections import deque

BF16 = ml_dtypes.bfloat16

HIDDEN = 1024
N_TOK = 2048
BATCH = 4
N_CORES = 8

_CACHE = {}


def _build_nc(D, N):
    """Build + compile the per-core Bass program.

    Per-core tensor shapes (DL = D // 2 local q/k/v width):
      xT  [D, N]  bf16   : x[b] transposed
      wqT/wkT/wvT [D, DL] bf16 : W[hs:hs+DL, :].T
      woT [DL, D] bf16   : Wo[:, hs:hs+DL].T
      bqt [128, DL//128] f32 : bq slice, chunked per partition
      o   [N, D]  f32    : partial output (host sums pairs)
    """
    import concourse.bacc as bacc
    import concourse.mybir as mybir
    import concourse.tile as tile
    from contextlib import ExitStack

    dt = mybir.dt
    P = 128
    DL = D // 2
    KC = D // P          # d_model chunks (8)
    MC = DL // P         # head pairs (4)
    NHL = DL // 64       # local heads (8)
    NT = N // P          # token tiles (16)
    PHW = N // 2         # i-phase width (1024)
    JT = NT              # j tiles (16)
    MW = 512             # matmul moving width (PSUM bank)

    nc = bacc.Bacc("TRN2", target_bir_lowering=False, debug=False)

    xT = nc.dram_tensor("xT", [D, N], dt.bfloat16, kind="ExternalInput")
    wqT = nc.dram_tensor("wqT", [D, DL], dt.bfloat16, kind="ExternalInput")
    wkT = nc.dram_tensor("wkT", [D, DL], dt.bfloat16, kind="ExternalInput")
    wvT = nc.dram_tensor("wvT", [D, DL], dt.bfloat16, kind="ExternalInput")
    woT = nc.dram_tensor("woT", [DL, D], dt.bfloat16, kind="ExternalInput")
    bqt = nc.dram_tensor("bqt", [P, MC], dt.float32, kind="ExternalInput")
    o = nc.dram_tensor("o", [N, D], dt.float32, kind="ExternalOutput")
    # tail token tiles are delivered as two bf16 partial sums (3-pair
    # partial + last-pair finisher) that the host adds
    o2 = nc.dram_tensor("o2", [N // 2, D], dt.bfloat16, kind="ExternalOutput")
    o3 = nc.dram_tensor("o3", [N // 2, D], dt.bfloat16, kind="ExternalOutput")

    with tile.TileContext(nc) as tc, ExitStack() as ctx:
        pers = ctx.enter_context(tc.tile_pool(name="pers", bufs=1))
        work = ctx.enter_context(tc.tile_pool(name="work", bufs=2))
        psc = ctx.enter_context(tc.tile_pool(name="psc", bufs=2, space="PSUM"))
        ppv = ctx.enter_context(tc.tile_pool(name="ppv", bufs=1, space="PSUM"))
        pfl = ctx.enter_context(tc.tile_pool(name="pfl", bufs=1, space="PSUM"))

        # ---- persistent SBUF tiles ----
        xt_t = [pers.tile([P, N], dt.bfloat16, name=f"xT{k}", tag=f"xT{k}") for k in range(KC)]
        wq_t = [pers.tile([P, DL], dt.bfloat16, name=f"wq{k}", tag=f"wq{k}") for k in range(KC)]
        wk_t = [pers.tile([P, DL], dt.bfloat16, name=f"wk{k}", tag=f"wk{k}") for k in range(KC)]
        wv_t = [pers.tile([P, DL], dt.bfloat16, name=f"wv{k}", tag=f"wv{k}") for k in range(KC)]
        wo_t = [pers.tile([P, D], dt.bfloat16, name=f"wo{m}", tag=f"wo{m}") for m in range(MC)]
        bq_t = pers.tile([P, MC], dt.float32, name="bqt_sb", tag="bqt")
        qa = [pers.tile([P, N], dt.bfloat16, name=f"qa{m}", tag=f"qa{m}") for m in range(MC)]
        ka = [pers.tile([P, N], dt.bfloat16, name=f"ka{m}", tag=f"ka{m}") for m in range(MC)]
        vp = [pers.tile([P, NHL * 65], dt.bfloat16, name=f"vp{t}", tag=f"vp{t}") for t in range(NT)]
        attn = [pers.tile([P, N], dt.bfloat16, name=f"attn{m}", tag=f"attn{m}") for m in range(MC)]

        # ---- input DMAs, ordered for earliest PE start ----
        # k-proj consumes (wk[k], xt[k]) progressively (k-outer loop), so
        # interleave those first; wq next (q-proj runs second), wv for
        # v_proj, wo only needed mid-era by out-proj fillers.
        for k in range(KC):
            nc.sync.dma_start(wk_t[k][:], wkT[k * P:(k + 1) * P, :])
            nc.sync.dma_start(xt_t[k][:], xT[k * P:(k + 1) * P, :])
        nc.sync.dma_start(bq_t[:], bqt[:, :])
        for k in range(KC):
            nc.sync.dma_start(wq_t[k][:], wqT[k * P:(k + 1) * P, :])
        for k in range(KC):
            nc.sync.dma_start(wv_t[k][:], wvT[k * P:(k + 1) * P, :])
        for m in range(MC):
            nc.sync.dma_start(wo_t[m][:], woT[m * P:(m + 1) * P, :])

        # Rotate [128, PHW] psum slots across the psc (bufs=2) and pfl
        # (bufs=1) pools so consecutive chains double-buffer during the
        # warm phase; during the era, fillers use only pfl.
        _rot = [0]

        def big_psum():
            _rot[0] += 1
            pool = pfl if _rot[0] % 3 == 0 else psc
            tag = "fill" if pool is pfl else "sc"
            return pool.tile([P, PHW], dt.float32, tag=tag, name=tag)

        def kq_chain(wt, m, half, bias, dst, ps=None):
            """One [128, PHW] k- or q-projection chain, k-outer so the
            first matmul needs only (w[0], xt[0])."""
            ps = ps if ps is not None else big_psum()
            n0 = half * PHW
            for k in range(KC):
                for s in range(0, PHW, MW):
                    nc.tensor.matmul(
                        out=ps[:, s:s + MW],
                        lhsT=wt[k][:, m * P:(m + 1) * P],
                        rhs=xt_t[k][:, n0 + s:n0 + s + MW],
                        start=(k == 0),
                        stop=(k == KC - 1),
                    )
            if bias is None:
                nc.vector.tensor_copy(dst[m][:, n0:n0 + PHW], ps[:])
            else:
                nc.vector.tensor_scalar_add(
                    dst[m][:, n0:n0 + PHW], ps[:], bias[:, m:m + 1])

        def v_tile(t, ps, s0):
            """v projection for token tile t into ps[:, s0:s0+DL]."""
            for k in range(KC):
                nc.tensor.matmul(
                    out=ps[:, s0:s0 + DL],
                    lhsT=xt_t[k][:, t * P:(t + 1) * P],
                    rhs=wv_t[k][:, :],
                    start=(k == 0),
                    stop=(k == KC - 1),
                )
            seg = vp[t][:].rearrange("p (s c) -> p s c", c=65)
            nc.vector.memset(seg[:, :, 64:65], 1.0)
            nc.vector.tensor_copy(
                seg[:, :, 0:64],
                ps[:, s0:s0 + DL].rearrange("p (s c) -> p s c", c=64),
            )

        # ---- warm phase: k/q proj for pairs 0-1, v projection t0-11 ----
        # (v t12-15 and kq pairs 2-3 become era fillers). Wave 1 runs three
        # k-outer chains jointly so the PE consumes each (wk[k], xt[k]) DMA
        # arrival with 6 matmuls instead of 2, staying near-dense while the
        # input stream lands.
        wave1 = [(0, 0), (0, 1), (1, 0)]
        w1ps = [big_psum() for _ in wave1]
        for k in range(KC):
            for (m, half), ps in zip(wave1, w1ps):
                n0 = half * PHW
                for s in range(0, PHW, MW):
                    nc.tensor.matmul(
                        out=ps[:, s:s + MW],
                        lhsT=wk_t[k][:, m * P:(m + 1) * P],
                        rhs=xt_t[k][:, n0 + s:n0 + s + MW],
                        start=(k == 0),
                        stop=(k == KC - 1),
                    )
        for (m, half), ps in zip(wave1, w1ps):
            nc.vector.tensor_copy(ka[m][:, half * PHW:half * PHW + PHW], ps[:])
        kq_chain(wk_t, 1, 1, None, ka)
        for m in (0, 1):
            for half in (0, 1):
                kq_chain(wq_t, m, half, bq_t, qa)
        for t in range(0, 12, 2):
            ps = big_psum()
            v_tile(t, ps, 0)
            v_tile(t + 1, ps, DL)

        # ---- filler machinery ----
        # Generators that emit ~2 matmuls per step; stepped between
        # attention ops to keep the PE dense while ScalarE runs exp.
        fillq = deque()
        fill_done = set()

        def fstep(n=1):
            for _ in range(n):
                while fillq:
                    try:
                        next(fillq[0][1])
                        break
                    except StopIteration:
                        fill_done.add(fillq[0][0])
                        fillq.popleft()

        def fdrain(name):
            # Emission-order deadline: Tile deps are versioned by emission
            # order, so a consumer emitted before the producer would read
            # stale data. Drain the queue (FIFO) until `name` completes.
            while name not in fill_done and fillq:
                try:
                    next(fillq[0][1])
                except StopIteration:
                    fill_done.add(fillq[0][0])
                    fillq.popleft()

        def v_filler():
            for tp in range(12, NT, 2):
                ps = pfl.tile([P, PHW], dt.float32, tag="fill", name="fill")
                for t in (tp, tp + 1):
                    s0 = (t - tp) * DL
                    for k in range(KC):
                        nc.tensor.matmul(
                            out=ps[:, s0:s0 + DL],
                            lhsT=xt_t[k][:, t * P:(t + 1) * P],
                            rhs=wv_t[k][:, :],
                            start=(k == 0),
                            stop=(k == KC - 1),
                        )
                        yield
                    seg = vp[t][:].rearrange("p (s c) -> p s c", c=65)
                    nc.vector.memset(seg[:, :, 64:65], 1.0)
                    nc.vector.tensor_copy(
                        seg[:, :, 0:64],
                        ps[:, s0:s0 + DL].rearrange("p (s c) -> p s c", c=64),
                    )
                    yield

        def kq_filler(m):
            for (wt, bias, dst) in ((wk_t, None, ka), (wq_t, bq_t, qa)):
                for half in (0, 1):
                    ps = pfl.tile([P, PHW], dt.float32, tag="fill", name="fill")
                    n0 = half * PHW
                    nmm = 0
                    for k in range(KC):
                        for s in range(0, PHW, MW):
                            nc.tensor.matmul(
                                out=ps[:, s:s + MW],
                                lhsT=wt[k][:, m * P:(m + 1) * P],
                                rhs=xt_t[k][:, n0 + s:n0 + s + MW],
                                start=(k == 0),
                                stop=(k == KC - 1),
                            )
                            yield
                    if bias is None:
                        nc.vector.tensor_copy(dst[m][:, n0:n0 + PHW], ps[:])
                    else:
                        nc.vector.tensor_scalar_add(
                            dst[m][:, n0:n0 + PHW], ps[:], bias[:, m:m + 1])
                    yield

        def outproj_filler(t, korder, partial_to=None):
            """Out-projection for token tile t over pairs `korder`.
            partial_to: bf16 SBUF tile for a 3-pair partial (tail path);
            None finishes to o via fp32 eviction + DMA."""
            ps = pfl.tile([P, D], dt.float32, tag="fill", name="fill")
            nmm = 0
            for ki, k in enumerate(korder):
                for s in range(0, D, MW):
                    nc.tensor.matmul(
                        out=ps[:, s:s + MW],
                        lhsT=attn[k][:, t * P:(t + 1) * P],
                        rhs=wo_t[k][:, s:s + MW],
                        start=(ki == 0),
                        stop=(ki == len(korder) - 1),
                    )
                    yield
            if partial_to is not None:
                t8 = t - NT // 2
                oe2 = work.tile([P, D], dt.bfloat16, tag="oe2", name="oe2",
                                bufs=2)
                nc.vector.tensor_copy(oe2[:], ps[:])
                nc.sync.dma_start(o2[t8 * P:(t8 + 1) * P, :], oe2[:])
            else:
                oe = work.tile([P, D], dt.float32, tag="oev", name="oe", bufs=3)
                nc.vector.tensor_copy(oe[:], ps[:])
                nc.sync.dma_start(o[t * P:(t + 1) * P, :], oe[:])
            yield

        def tail_finisher(t, klast):
            """Last pair's out-proj contribution, evicted on the (idle at
            tail) ScalarE to bf16 o3; the host adds o2 + o3."""
            ps = big_psum()
            t8 = t - NT // 2
            for s in range(0, D, MW):
                nc.tensor.matmul(
                    out=ps[:, s:s + MW],
                    lhsT=attn[klast][:, t * P:(t + 1) * P],
                    rhs=wo_t[klast][:, s:s + MW],
                    start=True,
                    stop=True,
                )
            oe3 = work.tile([P, D], dt.bfloat16, tag="oe3", name="oe3",
                            bufs=2)
            nc.scalar.activation(
                oe3[:], ps[:], mybir.ActivationFunctionType.Copy,
                bias=0.0, scale=1.0)
            nc.sync.dma_start(o3[t8 * P:(t8 + 1) * P, :], oe3[:])

        # ---- attention era: globally software-pipelined ----
        # Cycle c = (phase, head, j). The S+exp stream runs LAG cycles
        # ahead of the PV stream, so ScalarE accumulates an exp lead that
        # bridges filler-dry stretches and unit boundaries without ever
        # stalling the PE's PV matmuls.
        LAG = 8
        PTS_BUFS = LAG + 3
        cycles = [(phase, h, j)
                  for phase in (0, 1) for h in range(NHL) for j in range(JT)]
        NC_ = len(cycles)
        pts_of = {}
        pvt_of = {}

        def emit_scores(ci):
            phase, h, j = cycles[ci]
            m, r = h // 2, (h % 2) * 64
            i0 = phase * PHW
            sct = psc.tile([P, PHW], dt.float32, tag="sc", name="sc")
            for s in range(0, PHW, MW):
                nc.tensor.matmul(
                    out=sct[:, s:s + MW],
                    lhsT=ka[m][r:r + 64, j * P:(j + 1) * P],
                    rhs=qa[m][r:r + 64, i0 + s:i0 + s + MW],
                    start=True,
                    stop=True,
                )
            pts = work.tile([P, PHW], dt.bfloat16, tag="pts", name="pts",
                            bufs=PTS_BUFS)
            nc.scalar.activation(
                pts[:], sct[:],
                mybir.ActivationFunctionType.Exp,
                bias=0.0, scale=0.125,
            )
            pts_of[ci] = pts

        def emit_pv(ci):
            phase, h, j = cycles[ci]
            u = ci // JT
            if j == 0:
                pvt_of[u] = ppv.tile([65, PHW], dt.float32, tag="pv",
                                     name="pv")
            pvt = pvt_of[u]
            pts = pts_of.pop(ci)
            for s in range(0, PHW, MW):
                nc.tensor.matmul(
                    out=pvt[:, s:s + MW],
                    lhsT=vp[j][:, h * 65:(h + 1) * 65],
                    rhs=pts[:, s:s + MW],
                    start=(j == 0),
                    stop=(j == JT - 1),
                )
            if j == JT - 1:
                finish_unit(u, phase, h)

        def finish_unit(u, phase, h):
            # Evict PV fast (frees the single ppv buf), then the normalize
            # chain (all off the PE critical path). The custom-DVE
            # reciprocal ignores input partition offsets, so the denominator
            # row is first copied partition 64 -> 0 (tensor_copy handles the
            # shift) and the reciprocal runs at partition 0. Chunked
            # column-wise (4 chunks for the final unit, whose chain gates
            # the tail finishers) with full per-chunk tiles for the
            # custom-DVE reciprocal and the gpsimd broadcast, neither of
            # which handles AP offsets.
            m, r = h // 2, (h % 2) * 64
            i0 = phase * PHW
            pvt = pvt_of.pop(u)
            last = (u == 2 * NHL - 1)
            nch = 4 if last else 2
            cw = PHW // nch
            pv_sb = work.tile([64, PHW], dt.float32, tag="pvsb", name="pvsb",
                              bufs=2)
            if not last:
                # one full-width copy: fewest DVE ops gating the psum WAR
                # release for the next unit's PV j0
                nc.vector.tensor_copy(pv_sb[:], pvt[0:64, :])
            for ci in range(nch):
                sl = slice(ci * cw, (ci + 1) * cw)
                if last:
                    nc.vector.tensor_copy(pv_sb[:, sl], pvt[0:64, sl])
                den = work.tile([1, cw], dt.float32, tag=f"den{ci}{nch}",
                                name="den", bufs=1)
                nc.vector.tensor_copy(den[:], pvt[64:65, sl])
                recip = work.tile([1, cw], dt.float32, tag=f"recip{ci}{nch}",
                                  name="recip", bufs=1)
                nc.vector.reciprocal_approx_fast(recip[:], den[:])
                bcast = work.tile([64, cw], dt.float32, tag=f"bcast{ci}{nch}",
                                  name="bcast", bufs=1 if last else 2)
                nc.gpsimd.partition_broadcast(bcast[:], recip[:])
                nc.vector.tensor_tensor(
                    attn[m][r:r + 64, i0 + ci * cw:i0 + (ci + 1) * cw],
                    pv_sb[0:64, sl],
                    bcast[:],
                    mybir.AluOpType.mult,
                )

        fillq.append(("v", v_filler()))
        fillq.append(("kq2", kq_filler(2)))
        fillq.append(("kq3", kq_filler(3)))
        for ac in range(NC_ + LAG):
            if ac < NC_:
                phase, h, j = cycles[ac]
                if phase == 0 and j == 0:
                    if h == 4:
                        fdrain("kq2")
                    elif h == 6:
                        fdrain("kq3")
                emit_scores(ac)
            bc = ac - LAG
            if bc >= 0:
                if bc == 11:
                    fdrain("v")  # vp[12..15] must be emitted before PV j12+
                emit_pv(bc)
                if bc == NHL * JT - 1:
                    # all phase-0 normalizes emitted -> t0-7 out-projs
                    for t in range(NT // 2):
                        fillq.append((f"op{t}", outproj_filler(t, [0, 1, 2, 3])))
                if bc == NHL * JT + 6 * JT - 1:
                    # phase-1 heads 0-5 normalized -> 3-pair partials
                    for t in range(NT // 2, NT):
                        fillq.append((f"op{t}", outproj_filler(t, [0, 1, 2],
                                                               partial_to=True)))
            fstep(2 if ac % 3 == 0 else 1)
        # Tail: drain remaining fillers, then last-pair finishers.
        fstep(1000)
        for t in range(NT // 2, NT):
            tail_finisher(t, 3)

    nc.compile()
    return nc


def _get_nc(D, N):
    key = (D, N)
    if key not in _CACHE:
        _CACHE[key] = _build_nc(D, N)
    return _CACHE[key]


def _make_in_maps(x, Wq, bq, Wk, Wv, Wo, D, N):
    DL = D // 2
    MC = DL // 128
    in_maps = []
    for c in range(N_CORES):
        b = c // 2
        hs = (c % 2) * DL
        in_maps.append({
            "xT": np.ascontiguousarray(x[b].T).astype(BF16),
            "wqT": np.ascontiguousarray(Wq[hs:hs + DL, :].T).astype(BF16),
            "wkT": np.ascontiguousarray(Wk[hs:hs + DL, :].T).astype(BF16),
            "wvT": np.ascontiguousarray(Wv[hs:hs + DL, :].T).astype(BF16),
            "woT": np.ascontiguousarray(Wo[:, hs:hs + DL].T).astype(BF16),
            "bqt": np.ascontiguousarray(
                bq[hs:hs + DL].reshape(MC, 128).T).astype(np.float32),
        })
    return in_maps


def _run(x, Wq, bq, Wk, bk, Wv, bv, Wo, bo, trace=False):
    from concourse.bass_utils import run_bass_kernel_spmd

    x = np.asarray(x, np.float32)
    B, N, D = x.shape
    nc = _get_nc(D, N)
    in_maps = _make_in_maps(
        x, np.asarray(Wq, np.float32), np.asarray(bq, np.float32),
        np.asarray(Wk, np.float32), np.asarray(Wv, np.float32),
        np.asarray(Wo, np.float32), D, N)
    res = run_bass_kernel_spmd(
        nc, in_maps, list(range(N_CORES)), trace=trace)

    bv = np.asarray(bv, np.float32)
    bo = np.asarray(bo, np.float32)
    extra = bv @ np.asarray(Wo, np.float32).T + bo  # exact linear fold
    out = np.empty((B, N, D), np.float32)
    H2 = N // 2
    for b in range(B):
        for c in (2 * b, 2 * b + 1):
            r = res.results[c]
            part = r["o"].copy()
            part[H2:] = (np.asarray(r["o2"], np.float32)
                         + np.asarray(r["o3"], np.float32))
            out[b] = part if c == 2 * b else out[b] + part
        out[b] += extra
    return out, res


def kernel(x, Wq, bq, Wk, bk, Wv, bv, Wo, bo):
    out, _ = _run(x, Wq, bq, Wk, bk, Wv, bv, Wo, bo, trace=False)
    return out


# revision 23
# speedup vs baseline: 1.1924x; 1.1924x over previous
"""Multi-head self-attention Trainium2 kernel (8 NeuronCores).

Problem: B=4, N=2048, D=1024, H=16 heads of dim 64, fp32 in/out.

Sharding: 8 cores = 4 batches x 2 head-groups. Core c handles batch c//2
and heads (c%2)*8 .. (c%2)*8+7 (a 512-wide slice of the hidden dim).
Each core computes q/k/v projections for its head slice, attention for
its 8 heads, and a partial out-projection (contraction over its 512
attention dims). Host sums the two partials per batch.

Device dataflow (per core), all matmuls bf16 with fp32 PSUM accumulate:
  - x^T (host-pretransposed, bf16) in SBUF as 8 [128, 2048] tiles.
  - q_a/k_a in "layout a" [head_dim-part, token-free]; v in "layout b"
    [token-part, head_dim-free] restrided into per-head 65-col segments
    whose last column is ones (softmax denominator falls out of PV).
  - scores transposed: S^T[j, i] = k_a^T q_a (K=64), exp on ScalarE
    (scale 1/8 folded in; scores ~N(0,1) so no max subtraction), P^T
    bf16 straight to SBUF; PV accumulates [65, i] over 16 j-tiles.

Scheduling (the part that matters for speed): the PE p-state ramp
makes any tensor-engine idle gap doubly expensive (the next ~3us run
at half clock), and the ScalarE exp stream (33.5M elem -> ~280us) is
nearly as long as all matmul streams together (~330us), so the two
must overlap continuously. The kernel therefore runs 16 single-head
attention units (8 heads x 2 i-phases of 1024 tokens) sized so PSUM
splits into: scores [128,1024] x2 bufs (4 banks) + PV accum [65,1024]
x1 (2 banks) + a dedicated 2-bank "filler" pool. All remaining
projection / out-projection matmuls are emitted as fine-grained
fillers between attention ops, keeping the PE dense while ScalarE
holds a small exp lead. Input DMAs are ordered so the first projection
matmul can issue ~2us in (k-outer loops consume tiles as they land).

Normalize: reciprocal_approx_fast on the fp32 PV copy's denominator
row, partition-broadcast, multiply - off the critical path. Out-proj
for the last 8 token tiles accumulates pairs {0,1,2} early into bf16
partials; only the final pair's matmuls trail the last normalize.

Biases: bq on device; bk cancels in softmax; bv/bo folded on host.
"""

import numpy as np
import ml_dtypes
from collections import deque

BF16 = ml_dtypes.bfloat16

HIDDEN = 1024
N_TOK = 2048
BATCH = 4
N_CORES = 8

_CACHE = {}


def _build_nc(D, N):
    """Build + compile the per-core Bass program.

    Per-core tensor shapes (DL = D // 2 local q/k/v width):
      xT  [D, N]  bf16   : x[b] transposed
      wqT/wkT/wvT [D, DL] bf16 : W[hs:hs+DL, :].T
      woT [DL, D] bf16   : Wo[:, hs:hs+DL].T
      bqt [128, DL//128] f32 : bq slice, chunked per partition
      o   [N, D]  f32    : partial output (host sums pairs)
    """
    import concourse.bacc as bacc
    import concourse.mybir as mybir
    import concourse.tile as tile
    from contextlib import ExitStack

    dt = mybir.dt
    P = 128
    DL = D // 2
    KC = D // P          # d_model chunks (8)
    MC = DL // P         # head pairs (4)
    NHL = DL // 64       # local heads (8)
    NT = N // P          # token tiles (16)
    PHW = N // 2         # i-phase width (1024)
    JT = NT              # j tiles (16)
    MW = 512             # matmul moving width (PSUM bank)

    nc = bacc.Bacc("TRN2", target_bir_lowering=False, debug=False)

    xT = nc.dram_tensor("xT", [D, N], dt.bfloat16, kind="ExternalInput")
    wqT = nc.dram_tensor("wqT", [D, DL], dt.bfloat16, kind="ExternalInput")
    wkT = nc.dram_tensor("wkT", [D, DL], dt.bfloat16, kind="ExternalInput")
    wvT = nc.dram_tensor("wvT", [D, DL], dt.bfloat16, kind="ExternalInput")
    woT = nc.dram_tensor("woT", [DL, D], dt.bfloat16, kind="ExternalInput")
    bqt = nc.dram_tensor("bqt", [P, MC], dt.float32, kind="ExternalInput")
    o = nc.dram_tensor("o", [N, D], dt.float32, kind="ExternalOutput")
    # tail token tiles are delivered as two bf16 partial sums (3-pair
    # partial + last-pair finisher) that the host adds
    o2 = nc.dram_tensor("o2", [N // 2, D], dt.bfloat16, kind="ExternalOutput")
    o3 = nc.dram_tensor("o3", [N // 2, D], dt.bfloat16, kind="ExternalOutput")

    with tile.TileContext(nc) as tc, ExitStack() as ctx:
        pers = ctx.enter_context(tc.tile_pool(name="pers", bufs=1))
        work = ctx.enter_context(tc.tile_pool(name="work", bufs=2))
        psc = ctx.enter_context(tc.tile_pool(name="psc", bufs=2, space="PSUM"))
        ppv = ctx.enter_context(tc.tile_pool(name="ppv", bufs=1, space="PSUM"))
        pfl = ctx.enter_context(tc.tile_pool(name="pfl", bufs=1, space="PSUM"))

        # ---- persistent SBUF tiles ----
        xt_t = [pers.tile([P, N], dt.bfloat16, name=f"xT{k}", tag=f"xT{k}") for k in range(KC)]
        wq_t = [pers.tile([P, DL], dt.bfloat16, name=f"wq{k}", tag=f"wq{k}") for k in range(KC)]
        wk_t = [pers.tile([P, DL], dt.bfloat16, name=f"wk{k}", tag=f"wk{k}") for k in range(KC)]
        wv_t = [pers.tile([P, DL], dt.bfloat16, name=f"wv{k}", tag=f"wv{k}") for k in range(KC)]
        wo_t = [pers.tile([P, D], dt.bfloat16, name=f"wo{m}", tag=f"wo{m}") for m in range(MC)]
        bq_t = pers.tile([P, MC], dt.float32, name="bqt_sb", tag="bqt")
        qa = [pers.tile([P, N], dt.bfloat16, name=f"qa{m}", tag=f"qa{m}") for m in range(MC)]
        ka = [pers.tile([P, N], dt.bfloat16, name=f"ka{m}", tag=f"ka{m}") for m in range(MC)]
        vp = [pers.tile([P, NHL * 65], dt.bfloat16, name=f"vp{t}", tag=f"vp{t}") for t in range(NT)]
        attn = [pers.tile([P, N], dt.bfloat16, name=f"attn{m}", tag=f"attn{m}") for m in range(MC)]

        # ---- input DMAs, ordered for earliest PE start ----
        # k-proj consumes (wk[k], xt[k]) progressively (k-outer loop), so
        # interleave those first; wq next (q-proj runs second), wv for
        # v_proj, wo only needed mid-era by out-proj fillers.
        for k in range(KC):
            nc.sync.dma_start(wk_t[k][:], wkT[k * P:(k + 1) * P, :])
            nc.sync.dma_start(xt_t[k][:], xT[k * P:(k + 1) * P, :])
        nc.sync.dma_start(bq_t[:], bqt[:, :])
        for k in range(KC):
            nc.sync.dma_start(wq_t[k][:], wqT[k * P:(k + 1) * P, :])
        for k in range(KC):
            nc.sync.dma_start(wv_t[k][:], wvT[k * P:(k + 1) * P, :])
        for m in range(MC):
            nc.sync.dma_start(wo_t[m][:], woT[m * P:(m + 1) * P, :])

        # Rotate [128, PHW] psum slots across the psc (bufs=2) and pfl
        # (bufs=1) pools so consecutive chains double-buffer during the
        # warm phase; during the era, fillers use only pfl.
        _rot = [0]

        def big_psum():
            _rot[0] += 1
            pool = pfl if _rot[0] % 3 == 0 else psc
            tag = "fill" if pool is pfl else "sc"
            return pool.tile([P, PHW], dt.float32, tag=tag, name=tag)

        def kq_chain(wt, m, half, bias, dst, ps=None):
            """One [128, PHW] k- or q-projection chain, k-outer so the
            first matmul needs only (w[0], xt[0])."""
            ps = ps if ps is not None else big_psum()
            n0 = half * PHW
            for k in range(KC):
                for s in range(0, PHW, MW):
                    nc.tensor.matmul(
                        out=ps[:, s:s + MW],
                        lhsT=wt[k][:, m * P:(m + 1) * P],
                        rhs=xt_t[k][:, n0 + s:n0 + s + MW],
                        start=(k == 0),
                        stop=(k == KC - 1),
                    )
            if bias is None:
                nc.vector.tensor_copy(dst[m][:, n0:n0 + PHW], ps[:])
            else:
                nc.vector.tensor_scalar_add(
                    dst[m][:, n0:n0 + PHW], ps[:], bias[:, m:m + 1])

        def v_tile(t, ps, s0):
            """v projection for token tile t into ps[:, s0:s0+DL]."""
            for k in range(KC):
                nc.tensor.matmul(
                    out=ps[:, s0:s0 + DL],
                    lhsT=xt_t[k][:, t * P:(t + 1) * P],
                    rhs=wv_t[k][:, :],
                    start=(k == 0),
                    stop=(k == KC - 1),
                )
            seg = vp[t][:].rearrange("p (s c) -> p s c", c=65)
            nc.vector.memset(seg[:, :, 64:65], 1.0)
            nc.vector.tensor_copy(
                seg[:, :, 0:64],
                ps[:, s0:s0 + DL].rearrange("p (s c) -> p s c", c=64),
            )

        # ---- warm phase: k/q proj for pairs 0-1, v projection t0-11 ----
        # (v t12-15 and kq pairs 2-3 become era fillers). Wave 1 runs three
        # k-outer chains jointly so the PE consumes each (wk[k], xt[k]) DMA
        # arrival with 6 matmuls instead of 2, staying near-dense while the
        # input stream lands.
        wave1 = [(0, 0), (0, 1), (1, 0)]
        w1ps = [big_psum() for _ in wave1]
        for k in range(KC):
            for (m, half), ps in zip(wave1, w1ps):
                n0 = half * PHW
                for s in range(0, PHW, MW):
                    nc.tensor.matmul(
                        out=ps[:, s:s + MW],
                        lhsT=wk_t[k][:, m * P:(m + 1) * P],
                        rhs=xt_t[k][:, n0 + s:n0 + s + MW],
                        start=(k == 0),
                        stop=(k == KC - 1),
                    )
        for (m, half), ps in zip(wave1, w1ps):
            nc.vector.tensor_copy(ka[m][:, half * PHW:half * PHW + PHW], ps[:])
        kq_chain(wk_t, 1, 1, None, ka)
        for m in (0, 1):
            for half in (0, 1):
                kq_chain(wq_t, m, half, bq_t, qa)
        for t in range(0, 12, 2):
            ps = big_psum()
            v_tile(t, ps, 0)
            v_tile(t + 1, ps, DL)

        # ---- filler machinery ----
        # Generators that emit ~2 matmuls per step; stepped between
        # attention ops to keep the PE dense while ScalarE runs exp.
        fillq = deque()
        fill_done = set()

        def fstep(n=1):
            for _ in range(n):
                while fillq:
                    try:
                        next(fillq[0][1])
                        break
                    except StopIteration:
                        fill_done.add(fillq[0][0])
                        fillq.popleft()

        def fdrain(name):
            # Emission-order deadline: Tile deps are versioned by emission
            # order, so a consumer emitted before the producer would read
            # stale data. Drain the queue (FIFO) until `name` completes.
            while name not in fill_done and fillq:
                try:
                    next(fillq[0][1])
                except StopIteration:
                    fill_done.add(fillq[0][0])
                    fillq.popleft()

        def v_filler():
            for tp in range(12, NT, 2):
                ps = pfl.tile([P, PHW], dt.float32, tag="fill", name="fill")
                for t in (tp, tp + 1):
                    s0 = (t - tp) * DL
                    for k in range(KC):
                        nc.tensor.matmul(
                            out=ps[:, s0:s0 + DL],
                            lhsT=xt_t[k][:, t * P:(t + 1) * P],
                            rhs=wv_t[k][:, :],
                            start=(k == 0),
                            stop=(k == KC - 1),
                        )
                        yield
                    seg = vp[t][:].rearrange("p (s c) -> p s c", c=65)
                    nc.vector.memset(seg[:, :, 64:65], 1.0)
                    nc.vector.tensor_copy(
                        seg[:, :, 0:64],
                        ps[:, s0:s0 + DL].rearrange("p (s c) -> p s c", c=64),
                    )
                    yield

        def kq_filler(m):
            for (wt, bias, dst) in ((wk_t, None, ka), (wq_t, bq_t, qa)):
                for half in (0, 1):
                    ps = pfl.tile([P, PHW], dt.float32, tag="fill", name="fill")
                    n0 = half * PHW
                    nmm = 0
                    for k in range(KC):
                        for s in range(0, PHW, MW):
                            nc.tensor.matmul(
                                out=ps[:, s:s + MW],
                                lhsT=wt[k][:, m * P:(m + 1) * P],
                                rhs=xt_t[k][:, n0 + s:n0 + s + MW],
                                start=(k == 0),
                                stop=(k == KC - 1),
                            )
                            yield
                    if bias is None:
                        nc.vector.tensor_copy(dst[m][:, n0:n0 + PHW], ps[:])
                    else:
                        nc.vector.tensor_scalar_add(
                            dst[m][:, n0:n0 + PHW], ps[:], bias[:, m:m + 1])
                    yield

        def outproj_filler(t, korder, partial_to=None):
            """Out-projection for token tile t over pairs `korder`.
            partial_to: bf16 SBUF tile for a 3-pair partial (tail path);
            None finishes to o via fp32 eviction + DMA."""
            ps = pfl.tile([P, D], dt.float32, tag="fill", name="fill")
            nmm = 0
            for ki, k in enumerate(korder):
                for s in range(0, D, MW):
                    nc.tensor.matmul(
                        out=ps[:, s:s + MW],
                        lhsT=attn[k][:, t * P:(t + 1) * P],
                        rhs=wo_t[k][:, s:s + MW],
                        start=(ki == 0),
                        stop=(ki == len(korder) - 1),
                    )
                    yield
            if partial_to is not None:
                t8 = t - NT // 2
                oe2 = work.tile([P, D], dt.bfloat16, tag="oe2", name="oe2",
                                bufs=2)
                nc.vector.tensor_copy(oe2[:], ps[:])
                nc.sync.dma_start(o2[t8 * P:(t8 + 1) * P, :], oe2[:])
            else:
                oe = work.tile([P, D], dt.float32, tag="oev", name="oe", bufs=3)
                nc.vector.tensor_copy(oe[:], ps[:])
                nc.sync.dma_start(o[t * P:(t + 1) * P, :], oe[:])
            yield

        def tail_finisher(t, klast):
            """Last pair's out-proj contribution, evicted on the (idle at
            tail) ScalarE to bf16 o3; the host adds o2 + o3."""
            ps = big_psum()
            t8 = t - NT // 2
            for s in range(0, D, MW):
                nc.tensor.matmul(
                    out=ps[:, s:s + MW],
                    lhsT=attn[klast][:, t * P:(t + 1) * P],
                    rhs=wo_t[klast][:, s:s + MW],
                    start=True,
                    stop=True,
                )
            oe3 = work.tile([P, D], dt.bfloat16, tag="oe3", name="oe3",
                            bufs=2)
            nc.scalar.activation(
                oe3[:], ps[:], mybir.ActivationFunctionType.Copy,
                bias=0.0, scale=1.0)
            nc.sync.dma_start(o3[t8 * P:(t8 + 1) * P, :], oe3[:])

        # ---- attention era: globally software-pipelined ----
        # Cycle c = (phase, head, j). The S+exp stream runs LAG cycles
        # ahead of the PV stream, so ScalarE accumulates an exp lead that
        # bridges filler-dry stretches and unit boundaries without ever
        # stalling the PE's PV matmuls.
        LAG = 8
        PTS_BUFS = LAG + 3
        cycles = [(phase, h, j)
                  for phase in (0, 1) for h in range(NHL) for j in range(JT)]
        NC_ = len(cycles)
        pts_of = {}
        pvt_of = {}

        def emit_scores(ci):
            phase, h, j = cycles[ci]
            m, r = h // 2, (h % 2) * 64
            i0 = phase * PHW
            sct = psc.tile([P, PHW], dt.float32, tag="sc", name="sc")
            for s in range(0, PHW, MW):
                nc.tensor.matmul(
                    out=sct[:, s:s + MW],
                    lhsT=ka[m][r:r + 64, j * P:(j + 1) * P],
                    rhs=qa[m][r:r + 64, i0 + s:i0 + s + MW],
                    start=True,
                    stop=True,
                )
            pts = work.tile([P, PHW], dt.bfloat16, tag="pts", name="pts",
                            bufs=PTS_BUFS)
            nc.scalar.activation(
                pts[:], sct[:],
                mybir.ActivationFunctionType.Exp,
                bias=0.0, scale=0.125,
            )
            pts_of[ci] = pts

        def emit_pv(ci):
            phase, h, j = cycles[ci]
            u = ci // JT
            if j == 0:
                pvt_of[u] = ppv.tile([65, PHW], dt.float32, tag="pv",
                                     name="pv")
            pvt = pvt_of[u]
            pts = pts_of.pop(ci)
            for s in range(0, PHW, MW):
                nc.tensor.matmul(
                    out=pvt[:, s:s + MW],
                    lhsT=vp[j][:, h * 65:(h + 1) * 65],
                    rhs=pts[:, s:s + MW],
                    start=(j == 0),
                    stop=(j == JT - 1),
                )
            if j == JT - 1:
                finish_unit(u, phase, h)

        def finish_unit(u, phase, h):
            # Evict PV fast (frees the single ppv buf), then the normalize
            # chain (all off the PE critical path). The custom-DVE
            # reciprocal ignores input partition offsets, so the denominator
            # row is first copied partition 64 -> 0 (tensor_copy handles the
            # shift) and the reciprocal runs at partition 0. Chunked
            # column-wise (4 chunks for the final unit, whose chain gates
            # the tail finishers) with full per-chunk tiles for the
            # custom-DVE reciprocal and the gpsimd broadcast, neither of
            # which handles AP offsets.
            m, r = h // 2, (h % 2) * 64
            i0 = phase * PHW
            pvt = pvt_of.pop(u)
            last = (u == 2 * NHL - 1)
            nch = 4 if last else 2
            cw = PHW // nch
            # [65, PHW] eviction includes the denominator row, so the psum
            # WAR release for the next unit's PV j0 waits on ONE copy; the
            # den chunks then read SBUF off the release path.
            pv_sb = work.tile([65, PHW], dt.float32, tag="pvsb", name="pvsb",
                              bufs=2)
            if not last:
                nc.vector.tensor_copy(pv_sb[:], pvt[:])
            for ci in range(nch):
                sl = slice(ci * cw, (ci + 1) * cw)
                if last:
                    nc.vector.tensor_copy(pv_sb[:, sl], pvt[:, sl])
                den = work.tile([1, cw], dt.float32, tag=f"den{ci}{nch}",
                                name="den", bufs=1)
                nc.vector.tensor_copy(den[:], pv_sb[64:65, sl])
                recip = work.tile([1, cw], dt.float32, tag=f"recip{ci}{nch}",
                                  name="recip", bufs=1)
                nc.vector.reciprocal_approx_fast(recip[:], den[:])
                bcast = work.tile([64, cw], dt.float32, tag=f"bcast{ci}{nch}",
                                  name="bcast", bufs=1 if last else 2)
                nc.gpsimd.partition_broadcast(bcast[:], recip[:])
                nc.vector.tensor_tensor(
                    attn[m][r:r + 64, i0 + ci * cw:i0 + (ci + 1) * cw],
                    pv_sb[0:64, sl],
                    bcast[:],
                    mybir.AluOpType.mult,
                )

        fillq.append(("v", v_filler()))
        fillq.append(("kq2", kq_filler(2)))
        fillq.append(("kq3", kq_filler(3)))
        for ac in range(NC_ + LAG):
            if ac < NC_:
                phase, h, j = cycles[ac]
                if phase == 0 and j == 0:
                    if h == 4:
                        fdrain("kq2")
                    elif h == 6:
                        fdrain("kq3")
                emit_scores(ac)
            bc = ac - LAG
            if bc >= 0:
                if bc == 11:
                    fdrain("v")  # vp[12..15] must be emitted before PV j12+
                emit_pv(bc)
                if bc == NHL * JT - 1:
                    # all phase-0 normalizes emitted -> t0-7 out-projs
                    for t in range(NT // 2):
                        fillq.append((f"op{t}", outproj_filler(t, [0, 1, 2, 3])))
                if bc == NHL * JT + 6 * JT - 1:
                    # phase-1 heads 0-5 normalized -> 3-pair partials
                    for t in range(NT // 2, NT):
                        fillq.append((f"op{t}", outproj_filler(t, [0, 1, 2],
                                                               partial_to=True)))
            fstep(2 if ac % 3 == 0 else 1)
        # Tail: drain remaining fillers, then last-pair finishers.
        fstep(1000)
        for t in range(NT // 2, NT):
            tail_finisher(t, 3)

    nc.compile()
    return nc


def _get_nc(D, N):
    key = (D, N)
    if key not in _CACHE:
        _CACHE[key] = _build_nc(D, N)
    return _CACHE[key]


def _make_in_maps(x, Wq, bq, Wk, Wv, Wo, D, N):
    DL = D // 2
    MC = DL // 128
    in_maps = []
    for c in range(N_CORES):
        b = c // 2
        hs = (c % 2) * DL
        in_maps.append({
            "xT": np.ascontiguousarray(x[b].T).astype(BF16),
            "wqT": np.ascontiguousarray(Wq[hs:hs + DL, :].T).astype(BF16),
            "wkT": np.ascontiguousarray(Wk[hs:hs + DL, :].T).astype(BF16),
            "wvT": np.ascontiguousarray(Wv[hs:hs + DL, :].T).astype(BF16),
            "woT": np.ascontiguousarray(Wo[:, hs:hs + DL].T).astype(BF16),
            "bqt": np.ascontiguousarray(
                bq[hs:hs + DL].reshape(MC, 128).T).astype(np.float32),
        })
    return in_maps


def _run(x, Wq, bq, Wk, bk, Wv, bv, Wo, bo, trace=False):
    from concourse.bass_utils import run_bass_kernel_spmd

    x = np.asarray(x, np.float32)
    B, N, D = x.shape
    nc = _get_nc(D, N)
    in_maps = _make_in_maps(
        x, np.asarray(Wq, np.float32), np.asarray(bq, np.float32),
        np.asarray(Wk, np.float32), np.asarray(Wv, np.float32),
        np.asarray(Wo, np.float32), D, N)
    res = run_bass_kernel_spmd(
        nc, in_maps, list(range(N_CORES)), trace=trace)

    bv = np.asarray(bv, np.float32)
    bo = np.asarray(bo, np.float32)
    extra = bv @ np.asarray(Wo, np.float32).T + bo  # exact linear fold
    out = np.empty((B, N, D), np.float32)
    H2 = N // 2
    for b in range(B):
        for c in (2 * b, 2 * b + 1):
            r = res.results[c]
            part = r["o"].copy()
            part[H2:] = (np.asarray(r["o2"], np.float32)
                         + np.asarray(r["o3"], np.float32))
            out[b] = part if c == 2 * b else out[b] + part
        out[b] += extra
    return out, res


def kernel(x, Wq, bq, Wk, bk, Wv, bv, Wo, bo):
    out, _ = _run(x, Wq, bq, Wk, bk, Wv, bv, Wo, bo, trace=False)
    return out


# revision 24
# speedup vs baseline: 1.2083x; 1.0133x over previous
"""Multi-head self-attention Trainium2 kernel (8 NeuronCores).

Problem: B=4, N=2048, D=1024, H=16 heads of dim 64, fp32 in/out.

Sharding: 8 cores = 4 batches x 2 head-groups. Core c handles batch c//2
and heads (c%2)*8 .. (c%2)*8+7 (a 512-wide slice of the hidden dim).
Each core computes q/k/v projections for its head slice, attention for
its 8 heads, and a partial out-projection (contraction over its 512
attention dims). Host sums the two partials per batch.

Device dataflow (per core), all matmuls bf16 with fp32 PSUM accumulate:
  - x^T (host-pretransposed, bf16) in SBUF as 8 [128, 2048] tiles.
  - q_a/k_a in "layout a" [head_dim-part, token-free]; v in "layout b"
    [token-part, head_dim-free] restrided into per-head 65-col segments
    whose last column is ones (softmax denominator falls out of PV).
  - scores transposed: S^T[j, i] = k_a^T q_a (K=64), exp on ScalarE
    (scale 1/8 folded in; scores ~N(0,1) so no max subtraction), P^T
    bf16 straight to SBUF; PV accumulates [65, i] over 16 j-tiles.

Scheduling (the part that matters for speed): the PE p-state ramp
makes any tensor-engine idle gap doubly expensive (the next ~3us run
at half clock), and the ScalarE exp stream (33.5M elem -> ~280us) is
nearly as long as all matmul streams together (~330us), so the two
must overlap continuously. The kernel therefore runs 16 single-head
attention units (8 heads x 2 i-phases of 1024 tokens) sized so PSUM
splits into: scores [128,1024] x2 bufs (4 banks) + PV accum [65,1024]
x1 (2 banks) + a dedicated 2-bank "filler" pool. All remaining
projection / out-projection matmuls are emitted as fine-grained
fillers between attention ops, keeping the PE dense while ScalarE
holds a small exp lead. Input DMAs are ordered so the first projection
matmul can issue ~2us in (k-outer loops consume tiles as they land).

Normalize: reciprocal_approx_fast on the fp32 PV copy's denominator
row, partition-broadcast, multiply - off the critical path. Out-proj
for the last 8 token tiles accumulates pairs {0,1,2} early into bf16
partials; only the final pair's matmuls trail the last normalize.

Biases: bq on device; bk cancels in softmax; bv/bo folded on host.
"""

import numpy as np
import ml_dtypes
from collections import deque

BF16 = ml_dtypes.bfloat16

HIDDEN = 1024
N_TOK = 2048
BATCH = 4
N_CORES = 8

_CACHE = {}


def _build_nc(D, N):
    """Build + compile the per-core Bass program.

    Per-core tensor shapes (DL = D // 2 local q/k/v width):
      xT  [D, N]  bf16   : x[b] transposed
      wqT/wkT/wvT [D, DL] bf16 : W[hs:hs+DL, :].T
      woT [DL, D] bf16   : Wo[:, hs:hs+DL].T
      bqt [128, DL//128] f32 : bq slice, chunked per partition
      o   [N, D]  f32    : partial output (host sums pairs)
    """
    import concourse.bacc as bacc
    import concourse.mybir as mybir
    import concourse.tile as tile
    from contextlib import ExitStack

    dt = mybir.dt
    P = 128
    DL = D // 2
    KC = D // P          # d_model chunks (8)
    MC = DL // P         # head pairs (4)
    NHL = DL // 64       # local heads (8)
    NT = N // P          # token tiles (16)
    PHW = N // 2         # i-phase width (1024)
    JT = NT              # j tiles (16)
    MW = 512             # matmul moving width (PSUM bank)

    nc = bacc.Bacc("TRN2", target_bir_lowering=False, debug=False)

    xT = nc.dram_tensor("xT", [D, N], dt.bfloat16, kind="ExternalInput")
    wqT = nc.dram_tensor("wqT", [D, DL], dt.bfloat16, kind="ExternalInput")
    wkT = nc.dram_tensor("wkT", [D, DL], dt.bfloat16, kind="ExternalInput")
    wvT = nc.dram_tensor("wvT", [D, DL], dt.bfloat16, kind="ExternalInput")
    woT = nc.dram_tensor("woT", [DL, D], dt.bfloat16, kind="ExternalInput")
    bqt = nc.dram_tensor("bqt", [P, MC], dt.float32, kind="ExternalInput")
    o = nc.dram_tensor("o", [N, D], dt.float32, kind="ExternalOutput")
    # tail token tiles are delivered as two bf16 partial sums (3-pair
    # partial + last-pair finisher) that the host adds
    o2 = nc.dram_tensor("o2", [N // 2, D], dt.bfloat16, kind="ExternalOutput")
    o3 = nc.dram_tensor("o3", [N // 2, D], dt.bfloat16, kind="ExternalOutput")

    with tile.TileContext(nc) as tc, ExitStack() as ctx:
        pers = ctx.enter_context(tc.tile_pool(name="pers", bufs=1))
        work = ctx.enter_context(tc.tile_pool(name="work", bufs=2))
        psc = ctx.enter_context(tc.tile_pool(name="psc", bufs=2, space="PSUM"))
        ppv = ctx.enter_context(tc.tile_pool(name="ppv", bufs=1, space="PSUM"))
        pfl = ctx.enter_context(tc.tile_pool(name="pfl", bufs=1, space="PSUM"))

        # ---- persistent SBUF tiles ----
        xt_t = [pers.tile([P, N], dt.bfloat16, name=f"xT{k}", tag=f"xT{k}") for k in range(KC)]
        wq_t = [pers.tile([P, DL], dt.bfloat16, name=f"wq{k}", tag=f"wq{k}") for k in range(KC)]
        wk_t = [pers.tile([P, DL], dt.bfloat16, name=f"wk{k}", tag=f"wk{k}") for k in range(KC)]
        wv_t = [pers.tile([P, DL], dt.bfloat16, name=f"wv{k}", tag=f"wv{k}") for k in range(KC)]
        wo_t = [pers.tile([P, D], dt.bfloat16, name=f"wo{m}", tag=f"wo{m}") for m in range(MC)]
        bq_t = pers.tile([P, MC], dt.float32, name="bqt_sb", tag="bqt")
        qa = [pers.tile([P, N], dt.bfloat16, name=f"qa{m}", tag=f"qa{m}") for m in range(MC)]
        ka = [pers.tile([P, N], dt.bfloat16, name=f"ka{m}", tag=f"ka{m}") for m in range(MC)]
        vp = [pers.tile([P, NHL * 65], dt.bfloat16, name=f"vp{t}", tag=f"vp{t}") for t in range(NT)]
        attn = [pers.tile([P, N], dt.bfloat16, name=f"attn{m}", tag=f"attn{m}") for m in range(MC)]

        # ---- input DMAs, ordered for earliest PE start ----
        # k-proj consumes (wk[k], xt[k]) progressively (k-outer loop), so
        # interleave those first; wq next (q-proj runs second), wv for
        # v_proj, wo only needed mid-era by out-proj fillers.
        for k in range(KC):
            nc.sync.dma_start(wk_t[k][:], wkT[k * P:(k + 1) * P, :])
            nc.sync.dma_start(xt_t[k][:], xT[k * P:(k + 1) * P, :])
        nc.sync.dma_start(bq_t[:], bqt[:, :])
        for k in range(KC):
            nc.sync.dma_start(wq_t[k][:], wqT[k * P:(k + 1) * P, :])
        for k in range(KC):
            nc.sync.dma_start(wv_t[k][:], wvT[k * P:(k + 1) * P, :])
        for m in range(MC):
            nc.sync.dma_start(wo_t[m][:], woT[m * P:(m + 1) * P, :])

        # Rotate [128, PHW] psum slots across the psc (bufs=2) and pfl
        # (bufs=1) pools so consecutive chains double-buffer during the
        # warm phase; during the era, fillers use only pfl.
        _rot = [0]

        def big_psum():
            _rot[0] += 1
            pool = pfl if _rot[0] % 3 == 0 else psc
            tag = "fill" if pool is pfl else "sc"
            return pool.tile([P, PHW], dt.float32, tag=tag, name=tag)

        def kq_chain(wt, m, half, bias, dst, ps=None):
            """One [128, PHW] k- or q-projection chain, k-outer so the
            first matmul needs only (w[0], xt[0])."""
            ps = ps if ps is not None else big_psum()
            n0 = half * PHW
            for k in range(KC):
                for s in range(0, PHW, MW):
                    nc.tensor.matmul(
                        out=ps[:, s:s + MW],
                        lhsT=wt[k][:, m * P:(m + 1) * P],
                        rhs=xt_t[k][:, n0 + s:n0 + s + MW],
                        start=(k == 0),
                        stop=(k == KC - 1),
                    )
            if bias is None:
                nc.vector.tensor_copy(dst[m][:, n0:n0 + PHW], ps[:])
            else:
                nc.vector.tensor_scalar_add(
                    dst[m][:, n0:n0 + PHW], ps[:], bias[:, m:m + 1])

        def v_tile(t, ps, s0):
            """v projection for token tile t into ps[:, s0:s0+DL]."""
            for k in range(KC):
                nc.tensor.matmul(
                    out=ps[:, s0:s0 + DL],
                    lhsT=xt_t[k][:, t * P:(t + 1) * P],
                    rhs=wv_t[k][:, :],
                    start=(k == 0),
                    stop=(k == KC - 1),
                )
            seg = vp[t][:].rearrange("p (s c) -> p s c", c=65)
            nc.vector.memset(seg[:, :, 64:65], 1.0)
            nc.vector.tensor_copy(
                seg[:, :, 0:64],
                ps[:, s0:s0 + DL].rearrange("p (s c) -> p s c", c=64),
            )

        # ---- warm phase: k/q proj for pairs 0-1, v projection t0-11 ----
        # (v t12-15 and kq pairs 2-3 become era fillers). Wave 1 runs three
        # k-outer chains jointly so the PE consumes each (wk[k], xt[k]) DMA
        # arrival with 6 matmuls instead of 2, staying near-dense while the
        # input stream lands.
        wave1 = [(0, 0), (0, 1), (1, 0)]
        w1ps = [big_psum() for _ in wave1]
        for k in range(KC):
            for (m, half), ps in zip(wave1, w1ps):
                n0 = half * PHW
                for s in range(0, PHW, MW):
                    nc.tensor.matmul(
                        out=ps[:, s:s + MW],
                        lhsT=wk_t[k][:, m * P:(m + 1) * P],
                        rhs=xt_t[k][:, n0 + s:n0 + s + MW],
                        start=(k == 0),
                        stop=(k == KC - 1),
                    )
        for (m, half), ps in zip(wave1, w1ps):
            nc.vector.tensor_copy(ka[m][:, half * PHW:half * PHW + PHW], ps[:])
        kq_chain(wk_t, 1, 1, None, ka)
        for m in (0, 1):
            for half in (0, 1):
                kq_chain(wq_t, m, half, bq_t, qa)
        for t in range(0, 12, 2):
            ps = big_psum()
            v_tile(t, ps, 0)
            v_tile(t + 1, ps, DL)

        # ---- filler machinery ----
        # Generators that emit ~2 matmuls per step; stepped between
        # attention ops to keep the PE dense while ScalarE runs exp.
        fillq = deque()
        fill_done = set()

        def fstep(n=1):
            for _ in range(n):
                while fillq:
                    try:
                        next(fillq[0][1])
                        break
                    except StopIteration:
                        fill_done.add(fillq[0][0])
                        fillq.popleft()

        def fdrain(name):
            # Emission-order deadline: Tile deps are versioned by emission
            # order, so a consumer emitted before the producer would read
            # stale data. Drain the queue (FIFO) until `name` completes.
            while name not in fill_done and fillq:
                try:
                    next(fillq[0][1])
                except StopIteration:
                    fill_done.add(fillq[0][0])
                    fillq.popleft()

        def v_filler():
            for tp in range(12, NT, 2):
                ps = pfl.tile([P, PHW], dt.float32, tag="fill", name="fill")
                for t in (tp, tp + 1):
                    s0 = (t - tp) * DL
                    for k in range(KC):
                        nc.tensor.matmul(
                            out=ps[:, s0:s0 + DL],
                            lhsT=xt_t[k][:, t * P:(t + 1) * P],
                            rhs=wv_t[k][:, :],
                            start=(k == 0),
                            stop=(k == KC - 1),
                        )
                        yield
                    seg = vp[t][:].rearrange("p (s c) -> p s c", c=65)
                    nc.vector.memset(seg[:, :, 64:65], 1.0)
                    nc.vector.tensor_copy(
                        seg[:, :, 0:64],
                        ps[:, s0:s0 + DL].rearrange("p (s c) -> p s c", c=64),
                    )
                    yield

        def kq_filler(m):
            for (wt, bias, dst) in ((wk_t, None, ka), (wq_t, bq_t, qa)):
                for half in (0, 1):
                    ps = pfl.tile([P, PHW], dt.float32, tag="fill", name="fill")
                    n0 = half * PHW
                    nmm = 0
                    for k in range(KC):
                        for s in range(0, PHW, MW):
                            nc.tensor.matmul(
                                out=ps[:, s:s + MW],
                                lhsT=wt[k][:, m * P:(m + 1) * P],
                                rhs=xt_t[k][:, n0 + s:n0 + s + MW],
                                start=(k == 0),
                                stop=(k == KC - 1),
                            )
                            yield
                    if bias is None:
                        nc.vector.tensor_copy(dst[m][:, n0:n0 + PHW], ps[:])
                    else:
                        nc.vector.tensor_scalar_add(
                            dst[m][:, n0:n0 + PHW], ps[:], bias[:, m:m + 1])
                    yield

        def outproj_filler(t, korder, partial_to=None):
            """Out-projection for token tile t over pairs `korder`.
            partial_to: bf16 SBUF tile for a 3-pair partial (tail path);
            None finishes to o via fp32 eviction + DMA."""
            ps = pfl.tile([P, D], dt.float32, tag="fill", name="fill")
            nmm = 0
            for ki, k in enumerate(korder):
                for s in range(0, D, MW):
                    nc.tensor.matmul(
                        out=ps[:, s:s + MW],
                        lhsT=attn[k][:, t * P:(t + 1) * P],
                        rhs=wo_t[k][:, s:s + MW],
                        start=(ki == 0),
                        stop=(ki == len(korder) - 1),
                    )
                    yield
            if partial_to is not None:
                t8 = t - NT // 2
                oe2 = work.tile([P, D], dt.bfloat16, tag="oe2", name="oe2",
                                bufs=2)
                nc.vector.tensor_copy(oe2[:], ps[:])
                nc.sync.dma_start(o2[t8 * P:(t8 + 1) * P, :], oe2[:])
            else:
                oe = work.tile([P, D], dt.float32, tag="oev", name="oe", bufs=3)
                nc.vector.tensor_copy(oe[:], ps[:])
                nc.sync.dma_start(o[t * P:(t + 1) * P, :], oe[:])
            yield

        def tail_finisher(t, korder):
            """Late pairs' out-proj contribution to bf16 o3 (host adds
            o2 + o3). Evicted on DVE: ScalarE still owes queued exps at the
            tail, DVE is already drained."""
            ps = big_psum()
            t8 = t - NT // 2
            for ki, k in enumerate(korder):
                for s in range(0, D, MW):
                    nc.tensor.matmul(
                        out=ps[:, s:s + MW],
                        lhsT=attn[k][:, t * P:(t + 1) * P],
                        rhs=wo_t[k][:, s:s + MW],
                        start=(ki == 0),
                        stop=(ki == len(korder) - 1),
                    )
            oe3 = work.tile([P, D], dt.bfloat16, tag="oe3", name="oe3",
                            bufs=2)
            nc.vector.tensor_copy(oe3[:], ps[:])
            nc.sync.dma_start(o3[t8 * P:(t8 + 1) * P, :], oe3[:])

        # ---- attention era: globally software-pipelined ----
        # Cycle c = (phase, head, j). The S+exp stream runs LAG cycles
        # ahead of the PV stream, so ScalarE accumulates an exp lead that
        # bridges filler-dry stretches and unit boundaries without ever
        # stalling the PE's PV matmuls.
        LAG = 8
        PTS_BUFS = LAG + 3
        cycles = [(phase, h, j)
                  for phase in (0, 1) for h in range(NHL) for j in range(JT)]
        NC_ = len(cycles)
        pts_of = {}
        pvt_of = {}

        def emit_scores(ci):
            phase, h, j = cycles[ci]
            m, r = h // 2, (h % 2) * 64
            i0 = phase * PHW
            sct = psc.tile([P, PHW], dt.float32, tag="sc", name="sc")
            for s in range(0, PHW, MW):
                nc.tensor.matmul(
                    out=sct[:, s:s + MW],
                    lhsT=ka[m][r:r + 64, j * P:(j + 1) * P],
                    rhs=qa[m][r:r + 64, i0 + s:i0 + s + MW],
                    start=True,
                    stop=True,
                )
            pts = work.tile([P, PHW], dt.bfloat16, tag="pts", name="pts",
                            bufs=PTS_BUFS)
            nc.scalar.activation(
                pts[:], sct[:],
                mybir.ActivationFunctionType.Exp,
                bias=0.0, scale=0.125,
            )
            pts_of[ci] = pts

        def emit_pv(ci):
            phase, h, j = cycles[ci]
            u = ci // JT
            if j == 0:
                pvt_of[u] = ppv.tile([65, PHW], dt.float32, tag="pv",
                                     name="pv")
            pvt = pvt_of[u]
            pts = pts_of.pop(ci)
            for s in range(0, PHW, MW):
                nc.tensor.matmul(
                    out=pvt[:, s:s + MW],
                    lhsT=vp[j][:, h * 65:(h + 1) * 65],
                    rhs=pts[:, s:s + MW],
                    start=(j == 0),
                    stop=(j == JT - 1),
                )
            if j == JT - 1:
                finish_unit(u, phase, h)

        def finish_unit(u, phase, h):
            # Evict PV fast (frees the single ppv buf), then the normalize
            # chain (all off the PE critical path). The custom-DVE
            # reciprocal ignores input partition offsets, so the denominator
            # row is first copied partition 64 -> 0 (tensor_copy handles the
            # shift) and the reciprocal runs at partition 0. Chunked
            # column-wise (4 chunks for the final unit, whose chain gates
            # the tail finishers) with full per-chunk tiles for the
            # custom-DVE reciprocal and the gpsimd broadcast, neither of
            # which handles AP offsets.
            m, r = h // 2, (h % 2) * 64
            i0 = phase * PHW
            pvt = pvt_of.pop(u)
            last = (u == 2 * NHL - 1)
            nch = 4 if last else 2
            cw = PHW // nch
            # [65, PHW] eviction includes the denominator row, so the psum
            # WAR release for the next unit's PV j0 waits on ONE copy; the
            # den chunks then read SBUF off the release path.
            pv_sb = work.tile([65, PHW], dt.float32, tag="pvsb", name="pvsb",
                              bufs=2)
            if not last:
                nc.vector.tensor_copy(pv_sb[:], pvt[:])
            for ci in range(nch):
                sl = slice(ci * cw, (ci + 1) * cw)
                if last:
                    nc.vector.tensor_copy(pv_sb[:, sl], pvt[:, sl])
                den = work.tile([1, cw], dt.float32, tag=f"den{ci}{nch}",
                                name="den", bufs=1)
                nc.vector.tensor_copy(den[:], pv_sb[64:65, sl])
                recip = work.tile([1, cw], dt.float32, tag=f"recip{ci}{nch}",
                                  name="recip", bufs=1)
                nc.vector.reciprocal_approx_fast(recip[:], den[:])
                bcast = work.tile([64, cw], dt.float32, tag=f"bcast{ci}{nch}",
                                  name="bcast", bufs=1 if last else 2)
                nc.gpsimd.partition_broadcast(bcast[:], recip[:])
                nc.vector.tensor_tensor(
                    attn[m][r:r + 64, i0 + ci * cw:i0 + (ci + 1) * cw],
                    pv_sb[0:64, sl],
                    bcast[:],
                    mybir.AluOpType.mult,
                )

        fillq.append(("v", v_filler()))
        fillq.append(("kq2", kq_filler(2)))
        fillq.append(("kq3", kq_filler(3)))
        for ac in range(NC_ + LAG):
            if ac < NC_:
                phase, h, j = cycles[ac]
                if phase == 0 and j == 0:
                    if h == 4:
                        fdrain("kq2")
                    elif h == 6:
                        fdrain("kq3")
                emit_scores(ac)
            bc = ac - LAG
            if bc >= 0:
                if bc == 11:
                    fdrain("v")  # vp[12..15] must be emitted before PV j12+
                emit_pv(bc)
                if bc == NHL * JT - 1:
                    # all phase-0 normalizes emitted -> t0-7 out-projs
                    for t in range(NT // 2):
                        fillq.append((f"op{t}", outproj_filler(t, [0, 1, 2, 3])))
                if bc == NHL * JT + 4 * JT - 1:
                    # phase-1 heads 0-3 normalized -> 2-pair partials (early
                    # enqueue drains them inside the era, not at the tail)
                    for t in range(NT // 2, NT):
                        fillq.append((f"op{t}", outproj_filler(t, [0, 1],
                                                               partial_to=True)))
            fstep(2 if ac % 3 == 0 else 1)
        # Tail: drain remaining fillers, then last-pair finishers.
        fstep(1000)
        for t in range(NT // 2, NT):
            tail_finisher(t, [2, 3])

    nc.compile()
    return nc


def _get_nc(D, N):
    key = (D, N)
    if key not in _CACHE:
        _CACHE[key] = _build_nc(D, N)
    return _CACHE[key]


def _make_in_maps(x, Wq, bq, Wk, Wv, Wo, D, N):
    DL = D // 2
    MC = DL // 128
    in_maps = []
    for c in range(N_CORES):
        b = c // 2
        hs = (c % 2) * DL
        in_maps.append({
            "xT": np.ascontiguousarray(x[b].T).astype(BF16),
            "wqT": np.ascontiguousarray(Wq[hs:hs + DL, :].T).astype(BF16),
            "wkT": np.ascontiguousarray(Wk[hs:hs + DL, :].T).astype(BF16),
            "wvT": np.ascontiguousarray(Wv[hs:hs + DL, :].T).astype(BF16),
            "woT": np.ascontiguousarray(Wo[:, hs:hs + DL].T).astype(BF16),
            "bqt": np.ascontiguousarray(
                bq[hs:hs + DL].reshape(MC, 128).T).astype(np.float32),
        })
    return in_maps


def _run(x, Wq, bq, Wk, bk, Wv, bv, Wo, bo, trace=False):
    from concourse.bass_utils import run_bass_kernel_spmd

    x = np.asarray(x, np.float32)
    B, N, D = x.shape
    nc = _get_nc(D, N)
    in_maps = _make_in_maps(
        x, np.asarray(Wq, np.float32), np.asarray(bq, np.float32),
        np.asarray(Wk, np.float32), np.asarray(Wv, np.float32),
        np.asarray(Wo, np.float32), D, N)
    res = run_bass_kernel_spmd(
        nc, in_maps, list(range(N_CORES)), trace=trace)

    bv = np.asarray(bv, np.float32)
    bo = np.asarray(bo, np.float32)
    extra = bv @ np.asarray(Wo, np.float32).T + bo  # exact linear fold
    out = np.empty((B, N, D), np.float32)
    H2 = N // 2
    for b in range(B):
        for c in (2 * b, 2 * b + 1):
            r = res.results[c]
            part = r["o"].copy()
            part[H2:] = (np.asarray(r["o2"], np.float32)
                         + np.asarray(r["o3"], np.float32))
            out[b] = part if c == 2 * b else out[b] + part
        out[b] += extra
    return out, res


def kernel(x, Wq, bq, Wk, bk, Wv, bv, Wo, bo):
    out, _ = _run(x, Wq, bq, Wk, bk, Wv, bv, Wo, bo, trace=False)
    return out
